# revision 27
# baseline (speedup 1.0000x reference)
"""Trainium2 Bass kernel for nn_EnsemblePolicyHeads (MoE routing head).

Self-contained: accepts FULL inputs, shards batch across the 8 NeuronCores
(data parallel, weights replicated), returns the FULL [8192, 64] output.

v3: host pre-transposes/casts all operands to fp16 device layouts; device
kernel is a pure matmul pipeline.
  - z arrives as [ki=128, NT, KO, 512] fp16 (nt0 first, chunked so the
    logits + first-expert matmuls chase the DMA arrivals)
  - W1 as [ki=128, E, KO, H] fp16; Wa/W2/b1/b2 pre-laid-out
  - attn is normalized right after the logits (denominator via ones-matmul),
    broadcast across partitions on GpSimd; epilogue stores out^T and the
    host does the final [O,B] -> [B,O] transpose.
  - DMA instructions cost ~0.7us of issue time each, so inputs ride on two
    HWDGE queues (sync: z + W1, scalar: consts) in few, large transfers.
"""
import sys

for _p in ("/opt/trn_rl_repo",):
    if _p not in sys.path:
        sys.path.insert(0, _p)


import numpy as np
from contextlib import ExitStack

import concourse.bass as bass
import concourse.tile as tile
from concourse import bacc, mybir
from concourse.tile_rust import add_dep_helper

F32 = mybir.dt.float32
F16 = mybir.dt.float16
AF = mybir.ActivationFunctionType
ALU = mybir.AluOpType

D = 2048      # input dim
H = 128       # hidden
O = 64        # output dim
E = 16        # num experts
P = 128
KO = D // P   # 16 k-slices
NT_SIZE = 512
ZC = 4        # ko per z-nt0 chunk


def build_kernel(Bc: int):
    assert Bc % NT_SIZE == 0
    NT = Bc // NT_SIZE

    nc = bacc.Bacc("TRN2", target_bir_lowering=False, debug=False)
    zt_ap = nc.dram_tensor("zt", [P, NT, KO, NT_SIZE], F16,
                           kind="ExternalInput").ap()
    w1_ap = nc.dram_tensor("w1t", [P, E, KO, H], F16, kind="ExternalInput").ap()
    wa_ap = nc.dram_tensor("wat", [P, KO, E], F16, kind="ExternalInput").ap()
    w2_ap = nc.dram_tensor("w2t", [P, E, P], F16, kind="ExternalInput").ap()
    b1_ap = nc.dram_tensor("b1t", [P, E], F32, kind="ExternalInput").ap()
    b2_ap = nc.dram_tensor("b2t", [P, P], F16, kind="ExternalInput").ap()
    ba_ap = nc.dram_tensor("bat", [E, 1], F32, kind="ExternalInput").ap()
    rs_ap = nc.dram_tensor("rsel", [P, E, P], F16, kind="ExternalInput").ap()
    out_ap = nc.dram_tensor("out", [O, Bc], F32, kind="ExternalOutput").ap()

    with tile.TileContext(nc) as tc, ExitStack() as ctx:
        persist = ctx.enter_context(tc.tile_pool(name="persist", bufs=1))
        t_pool = ctx.enter_context(tc.tile_pool(name="t", bufs=3))
        hm_pool = ctx.enter_context(tc.tile_pool(name="hm", bufs=4))
        osb_pool = ctx.enter_context(tc.tile_pool(name="osb", bufs=2))
        psA = ctx.enter_context(tc.tile_pool(name="psA", bufs=2, space="PSUM"))
        psB = ctx.enter_context(tc.tile_pool(name="psB", bufs=2, space="PSUM"))
        psC = ctx.enter_context(tc.tile_pool(name="psC", bufs=2, space="PSUM"))
        psR = ctx.enter_context(tc.tile_pool(name="psR", bufs=2, space="PSUM"))

        # ---- persistent tiles ----
        zT = persist.tile([P, NT, KO, NT_SIZE], F16)
        w1sb = persist.tile([P, E, KO, H], F16)
        wasb = persist.tile([P, KO, E], F16)
        w2sb = persist.tile([P, E, P], F16)
        b1T = persist.tile([P, E], F32)
        b2sb = persist.tile([P, P], F16)
        ba_sb = persist.tile([E, 1], F32)
        ones16 = persist.tile([P, 1], F16)
        expT = persist.tile([P, Bc], F16)       # exp(logits) rows 0..15, zeros above
        recip = persist.tile([1, NT, NT_SIZE], F16)  # 1/denominator rows
        ones1_64 = persist.tile([1, O], F16)
        rec_sb = persist.tile([O, NT, NT_SIZE], F32)
        rsel_sb = persist.tile([P, E, P], F16)

        # ---- input DMAs.  All 8 cores pull inputs at once, so startup is
        # aggregate-HBM-bound: keep the early set minimal (z + first three
        # experts); the remaining W1 experts are paced by gating each DMA
        # on e-loop progress.  sync queue: W1 + z-nt0 chunks; scalar
        # queue: consts, then z-nt1 gated behind z-nt0. ----
        nc.scalar.dma_start(wasb[:], wa_ap[:])
        nc.scalar.dma_start(b1T[:], b1_ap[:])
        nc.scalar.dma_start(b2sb[:], b2_ap[:])
        nc.scalar.dma_start(ba_sb[:], ba_ap[:])
        nc.scalar.dma_start(w2sb[:], w2_ap[:])
        nc.scalar.dma_start(rsel_sb[:], rs_ap[:])

        def z_chunk(c):
            cs = slice(c * ZC, (c + 1) * ZC)
            return nc.sync.dma_start(zT[:, 0, cs], zt_ap[:, 0, cs])

        z_chunk(0)
        z_chunk(1)
        nc.sync.dma_start(w1sb[:, 0], w1_ap[:, 0])
        z_chunk(2)
        z_chunk(3)
        nc.sync.dma_start(w1sb[:, 1], w1_ap[:, 1])
        nc.sync.dma_start(w1sb[:, 2], w1_ap[:, 2])
        znt1_dmas = []
        for half in range(2):
            hs = slice(half * KO // 2, (half + 1) * KO // 2)
            for nt in range(1, NT):
                zd = nc.sync.dma_start(zT[:, nt, hs], zt_ap[:, nt, hs])
                znt1_dmas.append(zd)

        nc.vector.memset(ones16, 1.0)
        nc.vector.memset(expT[:], 0.0)
        nc.vector.memset(ones1_64, 1.0)
        ps_rec = {}

        # ---- PE warmup: ~3.5us of junk matmuls right after the preamble
        # so the HAM clock gate opens before the real stream begins ----
        junkw = persist.tile([P, P], F16)
        nc.vector.memset(junkw, 0.0)
        ps_ls = [psB.tile([E, NT_SIZE], F32, tag="ps_l", name=f"ps_l{nt}")
                 for nt in range(NT)]
        for _ in range(36):
            nc.tensor.matmul(ps_ls[0][:, :P], junkw[:, :E], junkw[:],
                             start=True, stop=True)

        def logits_softmax(nt):
            """exp -> stage -> broadcast (unnormalized; 1/denominator is
            applied at the epilogue, keeping this chain off the hm path)."""
            bs = slice(nt * NT_SIZE, (nt + 1) * NT_SIZE)
            nc.scalar.activation(expT[:E, bs], ps_ls[nt][:], AF.Exp,
                                 bias=ba_sb[:])
            # denominator for the epilogue (PE + DVE, tiny, not on hm path)
            ps_dn = psB.tile([1, NT_SIZE], F32, tag="ps_l", name=f"ps_dn{nt}")
            nc.tensor.matmul(ps_dn[:], ones16[:], expT[:, bs],
                             start=True, stop=True)
            with nc.allow_low_precision(reason="1/denom fits fp16"):
                nc.vector.reciprocal(recip[:, nt], ps_dn[:])


        # ---- main loop ----
        pend_w2 = []   # deferred (e, hm, ps_o, last) W2 matmuls

        def flush_w2(keep):
            while len(pend_w2) > keep:
                pe, phm, po, last = pend_w2.pop(0)
                nc.tensor.matmul(po[:], w2sb[:, pe, :], phm[:],
                                 start=False, stop=last)

        gidx = 0  # global e-group index, for W1 pacing
        for nt in range(NT):
            ps_o = psC.tile([P, NT_SIZE], F32)
            bs = slice(nt * NT_SIZE, (nt + 1) * NT_SIZE)
            for e in range(E):
                ps_h = psA.tile([P, NT_SIZE], F32)
                mm0 = None
                if nt == 0 and e == 0:
                    # chase the z-nt0 chunk arrivals: logits for chunks
                    # c and c+1 run before expert-0's W1 for chunk c, so
                    # the w1[0] transfer has time to land
                    NCH = KO // ZC

                    def logit_chunk(c):
                        m0 = None
                        for ko in range(c * ZC, (c + 1) * ZC):
                            m = nc.tensor.matmul(
                                ps_ls[0][:], wasb[:, ko, :], zT[:, 0, ko],
                                start=(ko == 0), stop=(ko == KO - 1))
                            m0 = m0 or m
                        return m0

                    def w1e0_chunk(c):
                        for ko in range(c * ZC, (c + 1) * ZC):
                            nc.tensor.matmul(
                                ps_h[:], w1sb[:, 0, ko, :], zT[:, 0, ko],
                                start=(ko == 0), stop=(ko == KO - 1))

                    mm0 = logit_chunk(0)
                    logit_chunk(1)
                    w1e0_chunk(0)
                    mlast = None
                    for c in range(2, NCH):
                        mlast = logit_chunk(c)
                        w1e0_chunk(c - 1)
                    w1e0_chunk(NCH - 1)
                    # release the z-nt1 transfers only once the chase has
                    # consumed z-nt0 (keeps startup HBM bandwidth for nt0)
                    for zd in znt1_dmas:
                        add_dep_helper(zd.ins, mlast.ins,
                                       reason="z-nt0 gets bw first")
                else:
                    for ko in range(KO):
                        m = nc.tensor.matmul(
                            ps_h[:], w1sb[:, e, ko, :], zT[:, nt, ko],
                            start=(ko == 0), stop=(ko == KO - 1))
                        mm0 = mm0 or m
                # paced W1 prefetch: expert gidx+3 loads once group gidx
                # has started issuing (keeps startup HBM free for z)
                if gidx + 3 < E:
                    wd = nc.sync.dma_start(w1sb[:, gidx + 3],
                                           w1_ap[:, gidx + 3])
                    add_dep_helper(wd.ins, mm0.ins, reason="pace W1")
                gidx += 1
                if nt == 0 and e == 0:
                    logits_softmax(0)
                if nt == 0 and e == 3:
                    # remaining nt logits (z-nt1 has landed by now)
                    for ko in range(KO):
                        for nt2 in range(1, NT):
                            nc.tensor.matmul(
                                ps_ls[nt2][:], wasb[:, ko, :], zT[:, nt2, ko],
                                start=(ko == 0), stop=(ko == KO - 1))
                    for nt2 in range(1, NT):
                        logits_softmax(nt2)
                if e == 2:
                    # b2 contribution (sum_e b2[e,o]*exp[e,b]) opens the
                    # ps_o accumulation, before the first W2 flush below,
                    # so the last deferred W2 matmul can carry stop=True
                    nc.tensor.matmul(ps_o[:], b2sb[:], expT[:, bs],
                                     start=True, stop=False)
                if e == 8:
                    # replicate 1/denominator across the O partitions for
                    # the epilogue (recip row is long since ready)
                    ps_rec[nt] = psB.tile([O, NT_SIZE], F32, tag="ps_l",
                                          name=f"ps_rec{nt}")
                    nc.tensor.matmul(ps_rec[nt][:], ones1_64[:],
                                     recip[:, nt], start=True, stop=True)
                    nc.scalar.copy(rec_sb[:, nt], ps_rec[nt][:])
                # replicate exp row e across all 128 partitions on the PE
                ps_r = psR.tile([P, NT_SIZE], F32)
                nc.tensor.matmul(ps_r[:], rsel_sb[:, e, :], expT[:, bs],
                                 start=True, stop=True)
                flush_w2(2 if e < E - 1 else 0)
                t = t_pool.tile([P, NT_SIZE], F16)
                nc.scalar.activation(t[:], ps_h[:], AF.Relu,
                                     bias=b1T[:, e:e + 1])
                hm = hm_pool.tile([P, NT_SIZE], F16)
                nc.vector.tensor_tensor(hm[:], t[:], ps_r[:], ALU.mult)
                pend_w2.append((e, hm, ps_o, e == E - 1))
            flush_w2(0)
            # epilogue: normalize by 1/denominator and store (no PE)
            osb = osb_pool.tile([O, NT_SIZE], F32)
            nc.vector.tensor_tensor(osb[:], ps_o[:O], rec_sb[:, nt], ALU.mult)
            nc.scalar.dma_start(out_ap[:, bs], osb[:])

    nc.compile()
    return nc


# ---------------------------------------------------------------------------
# Harness entry point
# ---------------------------------------------------------------------------
N_CORES = 8
B_TOTAL = 8192
BC = B_TOTAL // N_CORES
NT_TOTAL = BC // NT_SIZE

_nc_cache = {}


def _get_nc():
    if "nc" not in _nc_cache:
        _nc_cache["nc"] = build_kernel(BC)
    return _nc_cache["nc"]


def make_in_maps(z_i, W1, b1, W2, b2, Wa, ba):
    """Host-side prep: per-core transposed fp16 operand layouts."""
    z = np.asarray(z_i, dtype=np.float32).reshape(B_TOTAL, D)
    W1 = np.asarray(W1, dtype=np.float32)
    b1 = np.asarray(b1, dtype=np.float32)
    W2 = np.asarray(W2, dtype=np.float32)
    b2 = np.asarray(b2, dtype=np.float32)
    Wa = np.asarray(Wa, dtype=np.float32)
    ba = np.asarray(ba, dtype=np.float32)

    w1t = np.ascontiguousarray(
        W1.reshape(E, KO, P, H).transpose(2, 0, 1, 3).astype(np.float16))
    wat = np.ascontiguousarray(
        Wa.reshape(KO, P, E).transpose(1, 0, 2).astype(np.float16))
    w2t = np.zeros((P, E, P), dtype=np.float16)
    w2t[:, :, :O] = W2.transpose(1, 0, 2)
    b1t = np.ascontiguousarray(b1.T)
    b2t = np.zeros((P, P), dtype=np.float16)
    b2t[:E, :O] = b2
    bat = np.ascontiguousarray(ba.reshape(E, 1))
    rsel = np.zeros((P, E, P), dtype=np.float16)
    for e in range(E):
        rsel[e, e, :] = 1.0

    in_maps = []
    for c in range(N_CORES):
        zc = z[c * BC:(c + 1) * BC]
        # [ki, nt, ko, b512]
        zt = np.ascontiguousarray(
            zc.reshape(NT_TOTAL, NT_SIZE, KO, P).transpose(3, 0, 2, 1)
            .astype(np.float16))
        in_maps.append(dict(zt=zt, w1t=w1t, wat=wat, w2t=w2t,
                            b1t=b1t, b2t=b2t, bat=bat, rsel=rsel))
    return in_maps


def kernel(z_i, W1, b1, W2, b2, Wa, ba):
    from concourse.bass_utils import run_bass_kernel_spmd

    nc = _get_nc()
    in_maps = make_in_maps(z_i, W1, b1, W2, b2, Wa, ba)
    res = run_bass_kernel_spmd(nc, in_maps, core_ids=list(range(N_CORES)))
    outT = np.concatenate([res.results[c]["out"] for c in range(N_CORES)],
                          axis=1)
    return np.ascontiguousarray(outT.T)
